# revision 43
# baseline (speedup 1.0000x reference)
"""Trainium2 Bass kernel for nn_Attention_27994596836196.

GQA attention block (B=2, S=2048, HID=4096, 32 q heads / 8 kv groups,
rope, causal, out-projection), tensor-parallel over the 8 NeuronCores of
one TRN2 chip: core c owns q heads 4c..4c+3 and kv group c.  Each core
computes its heads' Q^T/K^T/V projections from a host-pretransposed
activation matrix, runs causal flash-style attention in a transposed
(keys-on-partitions) layout, and contracts its 512-row slice of w_o into
a full-size partial output; the host sums the eight partials
(collectives deliberately avoided: a collective in the NEFF measurably
slows every PE instruction by ~20%).

PE-stream optimizations over the first working version (863us -> ~733us):
- startup: host-prepped partition-major weight layouts, consumption-
  ordered k-chunked loads split across the sync/scalar HWDGE + gpsimd
  SWDGE rings; first matmul at ~14us instead of 46us.
- softmax denominator: p-tiles accumulated on the vector engine in bf16
  with ONE ones-matmul per (head, query-chunk) instead of one per key
  tile (-288 matmuls).
- w_o interleave: attention is exp(ACT)-paced, so w_o m-tile chains for
  the previous query chunk are hand-drained one per attention key-tile
  to keep the PE saturated; final drain rotates all freed PSUM pools.
- causal trim: diagonal key tiles compute only columns >= 128*dd.
- rope: q/k d-columns host-permuted to [evens; odds] so the pair-swap is
  two 64-partition SBUF->SBUF copies instead of a PE matmul per tile.

Self-contained: builds and runs via concourse (bass/tile) from
/opt/trn_rl_repo through bass_utils.run_bass_kernel_spmd on cores 0-7.
"""

import os
import sys

sys.path.insert(0, "/opt/trn_rl_repo")

import numpy as np
import ml_dtypes

# NTFF profiling hook shim: this agent image's antenv package lacks
# axon_hooks, which run_bass_kernel_spmd(trace=True) imports.  Harmless
# when tracing is off; registers the real hook when available.
try:
    import antenv.axon_hooks  # noqa: F401
except ImportError:
    import types

    _m = types.ModuleType("antenv.axon_hooks")
    _m._HOOK = None
    _m.set_axon_ntff_profile_hook = lambda h: setattr(_m, "_HOOK", h)
    _m.get_axon_ntff_profile_hook = lambda: _m._HOOK
    sys.modules["antenv.axon_hooks"] = _m
    try:
        import antenv

        antenv.axon_hooks = _m
        from trn_agent_boot.trn_boot import _ntff_profile_via_ctypes

        _m.set_axon_ntff_profile_hook(
            _ntff_profile_via_ctypes("/opt/axon/libaxon_pjrt.so")
        )
    except Exception:
        pass

import bass_rust
import concourse.bass as bass
import concourse.tile as tile
from concourse import bass_isa
from concourse import mybir
from concourse.bass_utils import run_bass_kernel_spmd
from contextlib import ExitStack

# ---------------------------------------------------------------------------
# Workaround for this walrus build's cap of ONE sync-wait command per
# instruction: Tile's sem-assignment freely attaches several waits to one
# instruction and codegen rejects it ("Too many sync wait commands").
# Split the waits across same-engine NoOps preceding the instruction.
# ---------------------------------------------------------------------------
MAX_WAITS = 1


def split_multi_waits(nc):
    n_split = 0
    for f in nc.m.functions:
        for bb in f.blocks:
            out = []
            for inst in bb.instructions:
                si = inst.sync_info
                if si is not None and si.on_wait and len(si.on_wait) > MAX_WAITS:
                    waits = list(si.on_wait)
                    extra, keep = waits[:-MAX_WAITS], waits[-MAX_WAITS:]
                    for i in range(0, len(extra), MAX_WAITS):
                        nop = bass_rust.InstNoOp(
                            name=f"I-{nc.next_id()}", ins=[], outs=[]
                        )
                        nop.engine = inst.engine
                        nop.sync_info = mybir.SyncInfo(
                            on_wait=extra[i : i + MAX_WAITS], on_update=[]
                        )
                        out.append(nop)
                    si.on_wait = keep
                    n_split += 1
                out.append(inst)
            bb.instructions[:] = out
    return n_split



BF16 = mybir.dt.bfloat16
F32 = mybir.dt.float32

N_CORES = 8
B, S, HID = 2, 2048, 4096
GKV = 8  # kv groups in the full model
BS = B * S  # 4096
D = 128
NH = 4          # q heads per core
KT = HID // 128  # 32 k-tiles
SC = 512        # free-dim chunk
NSC = BS // SC  # 8
SCALE = 1.0 / (D ** 0.5)
EXP = mybir.ActivationFunctionType.Exp
LOG = mybir.ActivationFunctionType.Ln


def build():
    nc = bass.Bass(num_devices=N_CORES)

    # Host-prepped layouts: partition dim first, per-partition contiguous
    # blocks so every HWDGE DMA uses >=1KB descriptors.
    xTc = nc.declare_dram_parameter("xTc", [128, NSC, KT, SC], BF16, isOutput=False)
    wq = nc.declare_dram_parameter("wq", [128, KT, NH * D], BF16, isOutput=False)
    wk = nc.declare_dram_parameter("wk", [128, KT, D], BF16, isOutput=False)
    wv = nc.declare_dram_parameter("wv", [128, KT, D], BF16, isOutput=False)
    wo = nc.declare_dram_parameter("wo", [128, NH, HID], BF16, isOutput=False)
    cosF = nc.declare_dram_parameter("cosF", [D, BS], BF16, isOutput=False)
    sinF = nc.declare_dram_parameter("sinF", [D, BS], BF16, isOutput=False)
    masks = nc.declare_dram_parameter("masks", [D, D], BF16, isOutput=False)
    outT = nc.declare_dram_parameter("outT", [HID, BS], BF16, isOutput=True)

    vT_d = nc.dram_tensor("vT_d", [D, BS], BF16)

    with tile.TileContext(nc, num_cores=N_CORES) as tc, ExitStack() as ctx:
        # ---- long-lived pools -------------------------------------------
        singles = ctx.enter_context(tc.tile_pool(name="singles", bufs=1))
        qkv_sb = ctx.enter_context(tc.tile_pool(name="qkv_sb", bufs=1))
        ps_acc = ctx.enter_context(tc.tile_pool(name="ps_acc", bufs=2, space="PSUM"))
        ps_s = ctx.enter_context(tc.tile_pool(name="ps_s", bufs=3, space="PSUM"))
        ps_l = ctx.enter_context(tc.tile_pool(name="ps_l", bufs=1, space="PSUM"))
        ps_w = ctx.enter_context(tc.tile_pool(name="ps_w", bufs=2, space="PSUM"))

        q_sb = [
            qkv_sb.tile([D, BS], BF16, tag=f"q{h}", name=f"q_sb{h}")
            for h in range(NH)
        ]
        k_sb = qkv_sb.tile([D, BS], BF16, tag="k")
        v_sb = qkv_sb.tile([D, KT, D], BF16, tag="v")  # V natural: [sk_local, j, d]

        # ---- phase 1: projections + rope --------------------------------
        with tc.tile_pool(name="w1", bufs=1) as w1, \
             tc.tile_pool(name="xt", bufs=4) as xtp, \
             tc.tile_pool(name="rope", bufs=4) as rope, \
             tc.tile_pool(name="vt", bufs=3) as vtp:

            # scalar HWDGE queue (idle engine in phase 1): weights in k-chunks,
            # in the order compute consumes them: wk (first projection), wv,
            # then the 4MB wq.
            wk_sb = w1.tile([128, KT, D], BF16, tag="wk")
            wv_sb = w1.tile([128, KT, D], BF16, tag="wv")
            wq_sb = w1.tile([128, KT, NH * D], BF16, tag="wq")
            for g in range(4):
                nc.scalar.dma_start(
                    wk_sb[:, g * 8:(g + 1) * 8, :], wk[:, g * 8:(g + 1) * 8, :]
                )
            for g in range(4):
                nc.scalar.dma_start(
                    wv_sb[:, g * 8:(g + 1) * 8, :], wv[:, g * 8:(g + 1) * 8, :]
                )
            for g in range(4):
                nc.scalar.dma_start(
                    wq_sb[:, g * 8:(g + 1) * 8, :], wq[:, g * 8:(g + 1) * 8, :]
                )
            # gpsimd SWDGE queue: rope tables (needed at the first rope_b,
            # which trails by a full t_i block) and the attention masks.
            cos_sb = singles.tile([D, BS], BF16)
            nc.gpsimd.dma_start(cos_sb[:], cosF[:])
            sin_sb = singles.tile([D, BS], BF16)
            nc.gpsimd.dma_start(sin_sb[:], sinF[:])
            mask_sb = singles.tile([D, D], BF16)
            nc.gpsimd.dma_start(mask_sb[:], masks[:])
            ones_sb = singles.tile([D, D], BF16)
            nc.vector.memset(ones_sb[:], 1.0)

            def rope_a(ps_q, cw):
                qeo = rope.tile([D, cw], BF16, tag="qeo")
                nc.vector.tensor_copy(qeo[:], ps_q[:])
                return qeo

            def rope_b(qeo, dst, col0, cw):
                # d-dims are host-permuted to [evens; odds], so the rope
                # pair-swap is an exchange of the two 64-partition halves:
                # two SBUF->SBUF DMA copies instead of a PE matmul.
                qsw = rope.tile([D, cw], BF16, tag="qsw")
                nc.sync.dma_start(qsw[0:64, :], qeo[64:128, :])
                nc.sync.dma_start(qsw[64:128, :], qeo[0:64, :])
                t1 = rope.tile([D, cw], BF16, tag="t1")
                nc.vector.tensor_mul(t1[:], qeo[:], cos_sb[:, col0:col0 + cw])
                t2 = rope.tile([D, cw], BF16, tag="t2")
                nc.vector.tensor_mul(t2[:], qsw[:], sin_sb[:, col0:col0 + cw])
                nc.vector.tensor_add(dst[:, col0:col0 + cw], t1[:], t2[:])

            def proj_pass(sc, c_lo, cw, nsub):
                # one projection pass over chunk sc's columns [c_lo, c_lo+cw)
                col0 = sc * SC + c_lo
                # sc=0 in fine subs on the sync ring; later chunks prefetch on
                # the scalar ring whose FIFO (wk, wv, wq, xt...) keeps startup
                # HBM bandwidth on the weights until they have landed.
                xt_dma = nc.sync.dma_start if sc == 0 else nc.scalar.dma_start
                xth = []
                for g in range(2):
                    t = xtp.tile([128, KT // 2, cw], BF16, tag="xt")
                    for q in range(nsub):
                        kn = (KT // 2) // nsub
                        xt_dma(
                            t[:, q * kn:(q + 1) * kn, :],
                            xTc[:, sc,
                                g * (KT // 2) + q * kn:
                                g * (KT // 2) + (q + 1) * kn,
                                c_lo:c_lo + cw],
                        )
                    xth.append(t)
                xts = [xth[k // (KT // 2)][:, k % (KT // 2), :] for k in range(KT)]

                # t_i order [k, v, q0..q3]: K first so compute starts on the
                # 1MB wk before the 4MB wq has streamed in.
                pending = None  # deferred rope_b so PE never waits on DVE copy
                for t_i in range(6):
                    ps_t = ps_acc.tile(
                        [D, cw], F32, tag="acc", name=f"ps_t{sc}_{c_lo}_{t_i}"
                    )
                    for k in range(KT):
                        if t_i == 0:
                            lhs = wk_sb[:, k, :]
                        elif t_i == 1:
                            lhs = wv_sb[:, k, :]
                        else:
                            lhs = wq_sb[:, k, bass.ts(t_i - 2, D)]
                        nc.tensor.matmul(
                            ps_t[:], lhs, xts[k],
                            start=(k == 0), stop=(k == KT - 1),
                        )
                    if t_i == 1:
                        vt = vtp.tile([D, cw], BF16, tag="vt")
                        nc.vector.tensor_copy(vt[:], ps_t[:])
                        nc.sync.dma_start(vT_d[:, col0:col0 + cw], vt[:])
                    else:
                        qeo = rope_a(ps_t, cw)
                        if pending is not None:
                            rope_b(*pending)
                        dst = k_sb if t_i == 0 else q_sb[t_i - 2]
                        pending = (qeo, dst, col0, cw)
                rope_b(*pending)

            for sc in range(NSC):
                proj_pass(sc, 0, SC, 4 if sc == 0 else 1)

                # V: read this chunk back transposed -> natural (sk, d) tiles
                # (sync queue: scalar engine must stay clear for phase-2 exps)
                for j in range(4 * sc, 4 * (sc + 1)):
                    nc.sync.dma_start_transpose(
                        v_sb[:, j, :], vT_d[:, bass.ts(j, D)]
                    )

        # ---- phase 2+3: attention with interleaved output projection ----
        # c-outer / h-inner: after all 4 heads finish query-chunk c, the
        # w_o matmuls for that chunk become runnable, and the scheduler
        # uses them to fill the PE while chunk c+1's attention is paced by
        # the scalar-engine exps.  The softmax denominator is accumulated
        # on the vector engine (bf16) with a single ones-matmul per chunk
        # instead of one per key tile.  Diagonal key tiles only compute
        # the columns the causal mask keeps (query cols >= 128*dd).
        with tc.tile_pool(name="pexp", bufs=8) as pexp, \
             tc.tile_pool(name="asml", bufs=2) as asml, \
             tc.tile_pool(name="paccp", bufs=3) as paccp, \
             tc.tile_pool(name="w3", bufs=1) as w3, \
             tc.tile_pool(name="aall", bufs=2) as aallp, \
             tc.tile_pool(name="o3p", bufs=4) as o3p:

            # sync ring, phase-2 priority: keeps the 4MB transfer out of the
            # startup HBM window (it would otherwise race xt/wq for bandwidth)
            wo_sb = w3.tile([128, NH, HID], BF16, tag="wo")
            for g in range(4):
                nc.sync.dma_start(
                    wo_sb[:, :, g * (HID // 4):(g + 1) * (HID // 4)],
                    wo[:, :, g * (HID // 4):(g + 1) * (HID // 4)],
                )

            # No collective: each core contracts only its own 4 heads' A^T
            # (512 of 4096 rows) against its w_o row-slice, producing a full
            # (HID, BS) partial that the host sums across cores.  The m-tile
            # emitters are queued and drained one per attention key-tile so
            # the PE always has w_o work while the exps pace attention.
            wo_fill = []
            fill_state = [0.0, 0.0]  # [accumulator, rate]

            def make_wo_m(b, nl, m, a_all):
                def emit(pool=ps_w, tg="wo"):
                    col = b * S + nl * SC
                    w_ps = pool.tile([D, SC], F32, tag=tg, name=f"wo{b}_{nl}_{m}")
                    for h in range(NH):
                        nc.tensor.matmul(
                            w_ps[:],
                            wo_sb[:, h, bass.ts(m, D)],
                            a_all[h][:, nl * SC:(nl + 1) * SC],
                            start=(h == 0), stop=(h == NH - 1),
                        )
                    ot = o3p.tile([D, SC], BF16, tag="ot", name=f"ot{b}_{nl}_{m}")
                    nc.any.tensor_copy(ot[:], w_ps[:])
                    nc.sync.dma_start(
                        outT[bass.ts(m, D), col:col + SC], ot[:]
                    )
                return emit

            def fill_tick():
                fill_state[0] += fill_state[1]
                while fill_state[0] >= 1.0 and wo_fill:
                    wo_fill.pop(0)()
                    fill_state[0] -= 1.0

            def attention_chunk(b, c, h, a_all):
                qh = q_sb[h]
                sq = b * S + c * SC
                nsk = 4 * (c + 1)
                o_ps = ps_acc.tile([D, SC], F32, tag="acc")
                pacc = paccp.tile([D, SC], BF16, tag="pacc")
                pend = []  # PE lookahead so o-matmuls trail the exps

                def flush(stop):
                    jp, pp, c0 = pend.pop(0)
                    nc.tensor.matmul(
                        o_ps[:, c0:], v_sb[:, b * (S // D) + jp, :], pp[:, c0:],
                        start=(jp == 0), stop=stop,
                    )

                for j in range(nsk):
                    dd = j - 4 * c
                    c0 = 128 * dd if dd > 0 else 0
                    s_ps = ps_s.tile([D, SC], F32, tag="s")
                    nc.tensor.matmul(
                        s_ps[:, c0:],
                        k_sb[:, b * S + j * D: b * S + (j + 1) * D],
                        qh[:, sq + c0:sq + SC],
                        start=True, stop=True,
                    )
                    if len(pend) == 4:
                        flush(False)
                    p_sb = pexp.tile([D, SC], BF16, tag="p")
                    nc.scalar.activation(p_sb[:, c0:], s_ps[:, c0:], EXP, scale=SCALE)
                    if dd >= 0:
                        # triangular mask on the leading 128 columns only
                        nc.vector.tensor_mul(
                            p_sb[:, c0:c0 + D], p_sb[:, c0:c0 + D],
                            mask_sb[:, 0:D],
                        )
                    if j == 0:
                        nc.vector.tensor_copy(pacc[:], p_sb[:])
                    else:
                        nc.vector.tensor_add(
                            pacc[:, c0:], pacc[:, c0:], p_sb[:, c0:]
                        )
                    pend.append((j, p_sb, c0))
                    fill_tick()
                while pend:
                    flush(len(pend) == 1)
                # denominator: single ones-matmul over the DVE-accumulated sum
                l_ps = ps_l.tile([D, SC], F32, tag="l")
                nc.tensor.matmul(l_ps[:], ones_sb[:], pacc[:], start=True, stop=True)
                # 1/l = exp(-ln(l)); ACT reciprocal is banned.
                lg = asml.tile([D, SC], F32, tag="lg")
                nc.scalar.activation(lg[:], l_ps[:], LOG)
                rec = asml.tile([D, SC], F32, tag="rec")
                nc.scalar.activation(rec[:], lg[:], EXP, scale=-1.0)
                nc.vector.tensor_mul(
                    a_all[h][:, c * SC:(c + 1) * SC], o_ps[:], rec[:]
                )

            for b in range(B):
                a_all = [
                    aallp.tile([D, S], BF16, tag=f"a{h}", name=f"a_all{b}_{h}")
                    for h in range(NH)
                ]
                for c in range(S // SC):
                    ntiles = 4 * (c + 1) * NH
                    # start draining a couple of tiles in: the previous
                    # chunk's last rescale is still in the ACT/DVE pipe
                    fill_state[0] = -2.0
                    fill_state[1] = len(wo_fill) / max(1, ntiles - 2)
                    for h in range(NH):
                        attention_chunk(b, c, h, a_all)
                    wo_fill.extend(
                        make_wo_m(b, c, m, a_all) for m in range(KT)
                    )
            # final drain: attention is done, so rotate through the freed
            # attention PSUM pools to avoid bank-reuse stalls
            drain_pools = [(ps_w, "wo"), (ps_s, "s"), (ps_acc, "acc"), (ps_l, "l")]
            di = 0
            while wo_fill:
                pool, tg = drain_pools[di % len(drain_pools)]
                di += 1
                wo_fill.pop(0)(pool, tg)

    split_multi_waits(nc)
    return nc


BF16_NP = ml_dtypes.bfloat16


def prep_inputs(x, cos_half, sin_half, w_q, w_k, w_v, w_o):
    x = np.asarray(x)
    cos_half = np.asarray(cos_half, dtype=np.float32)
    sin_half = np.asarray(sin_half, dtype=np.float32)
    w_q, w_k, w_v, w_o = (np.asarray(a) for a in (w_q, w_k, w_v, w_o))

    X = x.reshape(B * S, HID)
    xT = np.ascontiguousarray(X.T)  # (HID, BS) bf16
    # chunk-major layout: xTc[p, sc, k, s] = xT[p + 128k, sc*512 + s] so a
    # phase-1 tile DMA reads one contiguous 16-32KB block per partition
    xTc = np.ascontiguousarray(
        xT.reshape(KT, 128, NSC, SC).transpose(1, 2, 0, 3)
    )

    # d-dims of q/k are permuted to [evens; odds] (scores are invariant since
    # q and k share the permutation; V and w_o are untouched).  In that
    # layout rope's pair-swap is a swap of the 64-partition halves, and the
    # per-row tables are [cos; cos] and [-sin; sin].
    perm = np.concatenate([np.arange(0, D, 2), np.arange(1, D, 2)])
    w_q = np.ascontiguousarray(w_q.reshape(HID, HID // D, D)[:, :, perm].reshape(HID, HID))
    w_k = np.ascontiguousarray(w_k.reshape(HID, GKV, D)[:, :, perm].reshape(HID, GKV * D))

    cosb = cos_half.astype(BF16_NP)  # reference casts cos/sin to bf16 in _rope
    sinb = sin_half.astype(BF16_NP)
    cosF = np.ascontiguousarray(
        np.tile(np.vstack([cosb.T, cosb.T]), (1, B)), dtype=BF16_NP
    )
    sinF = np.ascontiguousarray(
        np.tile(np.vstack([-sinb.T, sinb.T]), (1, B)), dtype=BF16_NP
    )

    # triangular mask for the leading 128 columns of each diagonal tile
    p = np.arange(D)[:, None]
    f = np.arange(D)[None, :]
    masks = (f >= p).astype(BF16_NP)


    def pmajor(w):  # (4096, C) -> (128, KT_w, C) with row r = p + 128k
        kt = w.shape[0] // 128
        return np.ascontiguousarray(w.reshape(kt, 128, w.shape[1]).transpose(1, 0, 2))

    in_maps = []
    for c in range(N_CORES):
        in_maps.append(
            {
                "xTc": xTc,
                "wq": pmajor(w_q[:, c * 512:(c + 1) * 512]),
                "wk": pmajor(w_k[:, c * D:(c + 1) * D]),
                "wv": pmajor(w_v[:, c * D:(c + 1) * D]),
                "wo": pmajor(w_o[c * 512:(c + 1) * 512, :]),
                "cosF": cosF,
                "sinF": sinF,
                "masks": masks,
            }
        )
    return in_maps


def kernel(x, cos_half, sin_half, w_q, w_k, w_v, w_o, trace=None):
    if trace is None:
        trace = os.environ.get("KTRACE", "0") == "1"
    global LAST_RESULT
    in_maps = prep_inputs(x, cos_half, sin_half, w_q, w_k, w_v, w_o)
    res = run_bass_kernel_spmd(
        _nc(), in_maps, core_ids=list(range(N_CORES)), trace=trace
    )
    LAST_RESULT = res
    acc = res.results[0]["outT"].astype(np.float32)
    for c in range(1, N_CORES):
        acc += res.results[c]["outT"].astype(np.float32)
    return np.ascontiguousarray(acc.T).astype(BF16_NP).reshape(B, S, HID)


_NC = None
LAST_RESULT = None


def _nc():
    global _NC
    if _NC is None:
        _NC = build()
    return _NC



# revision 44
# speedup vs baseline: 1.1620x; 1.1620x over previous
"""Trainium2 Bass kernel for nn_Attention_27994596836196.

GQA attention block (B=2, S=2048, HID=4096, 32 q heads / 8 kv groups,
rope, causal, out-projection), tensor-parallel over the 8 NeuronCores of
one TRN2 chip: core c owns q heads 4c..4c+3 and kv group c.  Each core
computes its heads' Q^T/K^T/V projections from a host-pretransposed
activation matrix, runs causal flash-style attention in a transposed
(keys-on-partitions) layout, and contracts its 512-row slice of w_o into
a full-size partial output; the host sums the eight partials
(collectives deliberately avoided: a collective in the NEFF measurably
slows every PE instruction by ~20%).

PE-stream optimizations over the first working version (863us -> ~733us):
- startup: host-prepped partition-major weight layouts, consumption-
  ordered k-chunked loads split across the sync/scalar HWDGE + gpsimd
  SWDGE rings; first matmul at ~14us instead of 46us.
- softmax denominator: p-tiles accumulated on the vector engine in bf16
  with ONE ones-matmul per (head, query-chunk) instead of one per key
  tile (-288 matmuls).
- w_o interleave: attention is exp(ACT)-paced, so w_o m-tile chains for
  the previous query chunk are hand-drained one per attention key-tile
  to keep the PE saturated; final drain rotates all freed PSUM pools.
- causal trim: diagonal key tiles compute only columns >= 128*dd.
- rope: q/k d-columns host-permuted to [evens; odds] so the pair-swap is
  two 64-partition SBUF->SBUF copies instead of a PE matmul per tile.

Self-contained: builds and runs via concourse (bass/tile) from
/opt/trn_rl_repo through bass_utils.run_bass_kernel_spmd on cores 0-7.
"""

import os
import sys

sys.path.insert(0, "/opt/trn_rl_repo")

import numpy as np
import ml_dtypes

# NTFF profiling hook shim: this agent image's antenv package lacks
# axon_hooks, which run_bass_kernel_spmd(trace=True) imports.  Harmless
# when tracing is off; registers the real hook when available.
try:
    import antenv.axon_hooks  # noqa: F401
except ImportError:
    import types

    _m = types.ModuleType("antenv.axon_hooks")
    _m._HOOK = None
    _m.set_axon_ntff_profile_hook = lambda h: setattr(_m, "_HOOK", h)
    _m.get_axon_ntff_profile_hook = lambda: _m._HOOK
    sys.modules["antenv.axon_hooks"] = _m
    try:
        import antenv

        antenv.axon_hooks = _m
        from trn_agent_boot.trn_boot import _ntff_profile_via_ctypes

        _m.set_axon_ntff_profile_hook(
            _ntff_profile_via_ctypes("/opt/axon/libaxon_pjrt.so")
        )
    except Exception:
        pass

import bass_rust
import concourse.bass as bass
import concourse.tile as tile
from concourse import bass_isa
from concourse import mybir
from concourse.bass_utils import run_bass_kernel_spmd
from contextlib import ExitStack

# ---------------------------------------------------------------------------
# Workaround for this walrus build's cap of ONE sync-wait command per
# instruction: Tile's sem-assignment freely attaches several waits to one
# instruction and codegen rejects it ("Too many sync wait commands").
# Split the waits across same-engine NoOps preceding the instruction.
# ---------------------------------------------------------------------------
MAX_WAITS = 1


def split_multi_waits(nc):
    n_split = 0
    for f in nc.m.functions:
        for bb in f.blocks:
            out = []
            for inst in bb.instructions:
                si = inst.sync_info
                if si is not None and si.on_wait and len(si.on_wait) > MAX_WAITS:
                    waits = list(si.on_wait)
                    extra, keep = waits[:-MAX_WAITS], waits[-MAX_WAITS:]
                    for i in range(0, len(extra), MAX_WAITS):
                        nop = bass_rust.InstNoOp(
                            name=f"I-{nc.next_id()}", ins=[], outs=[]
                        )
                        nop.engine = inst.engine
                        nop.sync_info = mybir.SyncInfo(
                            on_wait=extra[i : i + MAX_WAITS], on_update=[]
                        )
                        out.append(nop)
                    si.on_wait = keep
                    n_split += 1
                out.append(inst)
            bb.instructions[:] = out
    return n_split



BF16 = mybir.dt.bfloat16
F32 = mybir.dt.float32

N_CORES = 8
B, S, HID = 2, 2048, 4096
GKV = 8  # kv groups in the full model
BS = B * S  # 4096
D = 128
NH = 4          # q heads per core
KT = HID // 128  # 32 k-tiles
SC = 512        # free-dim chunk
NSC = BS // SC  # 8
SCALE = 1.0 / (D ** 0.5)
EXP = mybir.ActivationFunctionType.Exp
LOG = mybir.ActivationFunctionType.Ln


def build():
    nc = bass.Bass(num_devices=N_CORES)

    # Host-prepped layouts: partition dim first, per-partition contiguous
    # blocks so every HWDGE DMA uses >=1KB descriptors.
    xTc = nc.declare_dram_parameter("xTc", [128, NSC, KT, SC], BF16, isOutput=False)
    wq = nc.declare_dram_parameter("wq", [128, KT, NH * D], BF16, isOutput=False)
    wk = nc.declare_dram_parameter("wk", [128, KT, D], BF16, isOutput=False)
    wv = nc.declare_dram_parameter("wv", [128, KT, D], BF16, isOutput=False)
    wo = nc.declare_dram_parameter("wo", [128, NH, HID], BF16, isOutput=False)
    cosF = nc.declare_dram_parameter("cosF", [D, BS], BF16, isOutput=False)
    sinF = nc.declare_dram_parameter("sinF", [D, BS], BF16, isOutput=False)
    masks = nc.declare_dram_parameter("masks", [D, D], BF16, isOutput=False)
    outT = nc.declare_dram_parameter("outT", [HID, BS], BF16, isOutput=True)

    vT_d = nc.dram_tensor("vT_d", [D, BS], BF16)

    with tile.TileContext(nc, num_cores=N_CORES) as tc, ExitStack() as ctx:
        # ---- long-lived pools -------------------------------------------
        singles = ctx.enter_context(tc.tile_pool(name="singles", bufs=1))
        qkv_sb = ctx.enter_context(tc.tile_pool(name="qkv_sb", bufs=1))
        ps_acc = ctx.enter_context(tc.tile_pool(name="ps_acc", bufs=2, space="PSUM"))
        ps_s = ctx.enter_context(tc.tile_pool(name="ps_s", bufs=3, space="PSUM"))
        ps_l = ctx.enter_context(tc.tile_pool(name="ps_l", bufs=1, space="PSUM"))
        ps_w = ctx.enter_context(tc.tile_pool(name="ps_w", bufs=2, space="PSUM"))

        q_sb = [
            qkv_sb.tile([D, BS], BF16, tag=f"q{h}", name=f"q_sb{h}")
            for h in range(NH)
        ]
        k_sb = qkv_sb.tile([D, BS], BF16, tag="k")
        v_sb = qkv_sb.tile([D, KT, D], BF16, tag="v")  # V natural: [sk_local, j, d]

        # ---- phase 1: projections + rope --------------------------------
        with tc.tile_pool(name="w1", bufs=1) as w1, \
             tc.tile_pool(name="xt", bufs=4) as xtp, \
             tc.tile_pool(name="rope", bufs=4) as rope, \
             tc.tile_pool(name="vt", bufs=3) as vtp:

            # scalar HWDGE queue (idle engine in phase 1): weights in k-chunks,
            # in the order compute consumes them: wk (first projection), wv,
            # then the 4MB wq.
            wk_sb = w1.tile([128, KT, D], BF16, tag="wk")
            wv_sb = w1.tile([128, KT, D], BF16, tag="wv")
            wq_sb = w1.tile([128, KT, NH * D], BF16, tag="wq")
            for g in range(4):
                nc.scalar.dma_start(
                    wk_sb[:, g * 8:(g + 1) * 8, :], wk[:, g * 8:(g + 1) * 8, :]
                )
            for g in range(4):
                nc.scalar.dma_start(
                    wv_sb[:, g * 8:(g + 1) * 8, :], wv[:, g * 8:(g + 1) * 8, :]
                )
            # wq split: first half after wk/wv on the scalar ring, second
            # half leads the gpsimd ring — balances the three DMA rings at
            # ~4MB each during the startup window.
            for g in range(2):
                nc.scalar.dma_start(
                    wq_sb[:, g * 8:(g + 1) * 8, :], wq[:, g * 8:(g + 1) * 8, :]
                )
            for g in range(2, 4):
                nc.gpsimd.dma_start(
                    wq_sb[:, g * 8:(g + 1) * 8, :], wq[:, g * 8:(g + 1) * 8, :]
                )
            # gpsimd SWDGE queue: rope tables (needed at the first rope_b,
            # which trails by a full t_i block) and the attention masks.
            cos_sb = singles.tile([D, BS], BF16)
            nc.gpsimd.dma_start(cos_sb[:], cosF[:])
            sin_sb = singles.tile([D, BS], BF16)
            nc.gpsimd.dma_start(sin_sb[:], sinF[:])
            mask_sb = singles.tile([D, D], BF16)
            nc.gpsimd.dma_start(mask_sb[:], masks[:])
            ones_sb = singles.tile([D, D], BF16)
            nc.vector.memset(ones_sb[:], 1.0)

            def rope_a(ps_q, cw):
                qeo = rope.tile([D, cw], BF16, tag="qeo")
                nc.vector.tensor_copy(qeo[:], ps_q[:])
                return qeo

            def rope_b(qeo, dst, col0, cw):
                # d-dims are host-permuted to [evens; odds], so the rope
                # pair-swap is an exchange of the two 64-partition halves:
                # two SBUF->SBUF DMA copies instead of a PE matmul.
                qsw = rope.tile([D, cw], BF16, tag="qsw")
                nc.sync.dma_start(qsw[0:64, :], qeo[64:128, :])
                nc.sync.dma_start(qsw[64:128, :], qeo[0:64, :])
                t1 = rope.tile([D, cw], BF16, tag="t1")
                nc.vector.tensor_mul(t1[:], qeo[:], cos_sb[:, col0:col0 + cw])
                t2 = rope.tile([D, cw], BF16, tag="t2")
                nc.vector.tensor_mul(t2[:], qsw[:], sin_sb[:, col0:col0 + cw])
                nc.vector.tensor_add(dst[:, col0:col0 + cw], t1[:], t2[:])

            def proj_pass(sc, c_lo, cw, nsub):
                # one projection pass over chunk sc's columns [c_lo, c_lo+cw)
                col0 = sc * SC + c_lo
                # sc=0 in fine subs on the sync ring; later chunks prefetch on
                # the scalar ring whose FIFO (wk, wv, wq, xt...) keeps startup
                # HBM bandwidth on the weights until they have landed.
                xt_dma = nc.sync.dma_start if sc == 0 else nc.scalar.dma_start
                xth = []
                for g in range(2):
                    t = xtp.tile([128, KT // 2, cw], BF16, tag="xt")
                    for q in range(nsub):
                        kn = (KT // 2) // nsub
                        xt_dma(
                            t[:, q * kn:(q + 1) * kn, :],
                            xTc[:, sc,
                                g * (KT // 2) + q * kn:
                                g * (KT // 2) + (q + 1) * kn,
                                c_lo:c_lo + cw],
                        )
                    xth.append(t)
                xts = [xth[k // (KT // 2)][:, k % (KT // 2), :] for k in range(KT)]

                # t_i order [k, v, q0..q3]: K first so compute starts on the
                # 1MB wk before the 4MB wq has streamed in.
                pending = None  # deferred rope_b so PE never waits on DVE copy
                for t_i in range(6):
                    ps_t = ps_acc.tile(
                        [D, cw], F32, tag="acc", name=f"ps_t{sc}_{c_lo}_{t_i}"
                    )
                    for k in range(KT):
                        if t_i == 0:
                            lhs = wk_sb[:, k, :]
                        elif t_i == 1:
                            lhs = wv_sb[:, k, :]
                        else:
                            lhs = wq_sb[:, k, bass.ts(t_i - 2, D)]
                        nc.tensor.matmul(
                            ps_t[:], lhs, xts[k],
                            start=(k == 0), stop=(k == KT - 1),
                        )
                    if t_i == 1:
                        vt = vtp.tile([D, cw], BF16, tag="vt")
                        nc.vector.tensor_copy(vt[:], ps_t[:])
                        nc.sync.dma_start(vT_d[:, col0:col0 + cw], vt[:])
                    else:
                        qeo = rope_a(ps_t, cw)
                        if pending is not None:
                            rope_b(*pending)
                        dst = k_sb if t_i == 0 else q_sb[t_i - 2]
                        pending = (qeo, dst, col0, cw)
                rope_b(*pending)

            for sc in range(NSC):
                proj_pass(sc, 0, SC, 4 if sc == 0 else 1)

                # V: read this chunk back transposed -> natural (sk, d) tiles
                # (sync queue: scalar engine must stay clear for phase-2 exps)
                for j in range(4 * sc, 4 * (sc + 1)):
                    nc.sync.dma_start_transpose(
                        v_sb[:, j, :], vT_d[:, bass.ts(j, D)]
                    )

        # ---- phase 2+3: attention with interleaved output projection ----
        # c-outer / h-inner: after all 4 heads finish query-chunk c, the
        # w_o matmuls for that chunk become runnable, and the scheduler
        # uses them to fill the PE while chunk c+1's attention is paced by
        # the scalar-engine exps.  The softmax denominator is accumulated
        # on the vector engine (bf16) with a single ones-matmul per chunk
        # instead of one per key tile.  Diagonal key tiles only compute
        # the columns the causal mask keeps (query cols >= 128*dd).
        with tc.tile_pool(name="pexp", bufs=8) as pexp, \
             tc.tile_pool(name="asml", bufs=2) as asml, \
             tc.tile_pool(name="paccp", bufs=3) as paccp, \
             tc.tile_pool(name="w3", bufs=1) as w3, \
             tc.tile_pool(name="aall", bufs=2) as aallp, \
             tc.tile_pool(name="o3p", bufs=4) as o3p:

            # sync ring, phase-2 priority: keeps the 4MB transfer out of the
            # startup HBM window (it would otherwise race xt/wq for bandwidth)
            wo_sb = w3.tile([128, NH, HID], BF16, tag="wo")
            for g in range(4):
                nc.sync.dma_start(
                    wo_sb[:, :, g * (HID // 4):(g + 1) * (HID // 4)],
                    wo[:, :, g * (HID // 4):(g + 1) * (HID // 4)],
                )

            # No collective: each core contracts only its own 4 heads' A^T
            # (512 of 4096 rows) against its w_o row-slice, producing a full
            # (HID, BS) partial that the host sums across cores.  The m-tile
            # emitters are queued and drained one per attention key-tile so
            # the PE always has w_o work while the exps pace attention.
            wo_fill = []
            fill_state = [0.0, 0.0]  # [accumulator, rate]

            def make_wo_m(b, nl, m, a_all):
                def emit(pool=ps_w, tg="wo"):
                    col = b * S + nl * SC
                    w_ps = pool.tile([D, SC], F32, tag=tg, name=f"wo{b}_{nl}_{m}")
                    for h in range(NH):
                        nc.tensor.matmul(
                            w_ps[:],
                            wo_sb[:, h, bass.ts(m, D)],
                            a_all[h][:, nl * SC:(nl + 1) * SC],
                            start=(h == 0), stop=(h == NH - 1),
                        )
                    ot = o3p.tile([D, SC], BF16, tag="ot", name=f"ot{b}_{nl}_{m}")
                    nc.any.tensor_copy(ot[:], w_ps[:])
                    nc.sync.dma_start(
                        outT[bass.ts(m, D), col:col + SC], ot[:]
                    )
                return emit

            def fill_tick():
                fill_state[0] += fill_state[1]
                while fill_state[0] >= 1.0 and wo_fill:
                    wo_fill.pop(0)()
                    fill_state[0] -= 1.0

            def attention_chunk(b, c, h, a_all):
                qh = q_sb[h]
                sq = b * S + c * SC
                nsk = 4 * (c + 1)
                o_ps = ps_acc.tile([D, SC], F32, tag="acc")
                pacc = paccp.tile([D, SC], BF16, tag="pacc")
                pend = []  # PE lookahead so o-matmuls trail the exps

                def flush(stop):
                    jp, pp, c0 = pend.pop(0)
                    nc.tensor.matmul(
                        o_ps[:, c0:], v_sb[:, b * (S // D) + jp, :], pp[:, c0:],
                        start=(jp == 0), stop=stop,
                    )

                for j in range(nsk):
                    dd = j - 4 * c
                    c0 = 128 * dd if dd > 0 else 0
                    s_ps = ps_s.tile([D, SC], F32, tag="s")
                    nc.tensor.matmul(
                        s_ps[:, c0:],
                        k_sb[:, b * S + j * D: b * S + (j + 1) * D],
                        qh[:, sq + c0:sq + SC],
                        start=True, stop=True,
                    )
                    if len(pend) == 4:
                        flush(False)
                    p_sb = pexp.tile([D, SC], BF16, tag="p")
                    nc.scalar.activation(p_sb[:, c0:], s_ps[:, c0:], EXP, scale=SCALE)
                    if dd >= 0:
                        # triangular mask on the leading 128 columns only
                        nc.vector.tensor_mul(
                            p_sb[:, c0:c0 + D], p_sb[:, c0:c0 + D],
                            mask_sb[:, 0:D],
                        )
                    if j == 0:
                        nc.vector.tensor_copy(pacc[:], p_sb[:])
                    else:
                        nc.vector.tensor_add(
                            pacc[:, c0:], pacc[:, c0:], p_sb[:, c0:]
                        )
                    pend.append((j, p_sb, c0))
                    fill_tick()
                while pend:
                    flush(len(pend) == 1)
                # denominator: single ones-matmul over the DVE-accumulated sum
                l_ps = ps_l.tile([D, SC], F32, tag="l")
                nc.tensor.matmul(l_ps[:], ones_sb[:], pacc[:], start=True, stop=True)
                # 1/l = exp(-ln(l)); ACT reciprocal is banned.
                lg = asml.tile([D, SC], F32, tag="lg")
                nc.scalar.activation(lg[:], l_ps[:], LOG)
                rec = asml.tile([D, SC], F32, tag="rec")
                nc.scalar.activation(rec[:], lg[:], EXP, scale=-1.0)
                nc.vector.tensor_mul(
                    a_all[h][:, c * SC:(c + 1) * SC], o_ps[:], rec[:]
                )

            for b in range(B):
                a_all = [
                    aallp.tile([D, S], BF16, tag=f"a{h}", name=f"a_all{b}_{h}")
                    for h in range(NH)
                ]
                for c in range(S // SC):
                    ntiles = 4 * (c + 1) * NH
                    # start draining a couple of tiles in: the previous
                    # chunk's last rescale is still in the ACT/DVE pipe
                    fill_state[0] = -2.0
                    fill_state[1] = len(wo_fill) / max(1, ntiles - 2)
                    for h in range(NH):
                        attention_chunk(b, c, h, a_all)
                    wo_fill.extend(
                        make_wo_m(b, c, m, a_all) for m in range(KT)
                    )
            # final drain: attention is done, so rotate through the freed
            # attention PSUM pools to avoid bank-reuse stalls
            drain_pools = [(ps_w, "wo"), (ps_s, "s"), (ps_acc, "acc"), (ps_l, "l")]
            di = 0
            while wo_fill:
                pool, tg = drain_pools[di % len(drain_pools)]
                di += 1
                wo_fill.pop(0)(pool, tg)

    split_multi_waits(nc)
    return nc


BF16_NP = ml_dtypes.bfloat16


def prep_inputs(x, cos_half, sin_half, w_q, w_k, w_v, w_o):
    x = np.asarray(x)
    cos_half = np.asarray(cos_half, dtype=np.float32)
    sin_half = np.asarray(sin_half, dtype=np.float32)
    w_q, w_k, w_v, w_o = (np.asarray(a) for a in (w_q, w_k, w_v, w_o))

    X = x.reshape(B * S, HID)
    xT = np.ascontiguousarray(X.T)  # (HID, BS) bf16
    # chunk-major layout: xTc[p, sc, k, s] = xT[p + 128k, sc*512 + s] so a
    # phase-1 tile DMA reads one contiguous 16-32KB block per partition
    xTc = np.ascontiguousarray(
        xT.reshape(KT, 128, NSC, SC).transpose(1, 2, 0, 3)
    )

    # d-dims of q/k are permuted to [evens; odds] (scores are invariant since
    # q and k share the permutation; V and w_o are untouched).  In that
    # layout rope's pair-swap is a swap of the 64-partition halves, and the
    # per-row tables are [cos; cos] and [-sin; sin].
    perm = np.concatenate([np.arange(0, D, 2), np.arange(1, D, 2)])
    w_q = np.ascontiguousarray(w_q.reshape(HID, HID // D, D)[:, :, perm].reshape(HID, HID))
    w_k = np.ascontiguousarray(w_k.reshape(HID, GKV, D)[:, :, perm].reshape(HID, GKV * D))

    cosb = cos_half.astype(BF16_NP)  # reference casts cos/sin to bf16 in _rope
    sinb = sin_half.astype(BF16_NP)
    cosF = np.ascontiguousarray(
        np.tile(np.vstack([cosb.T, cosb.T]), (1, B)), dtype=BF16_NP
    )
    sinF = np.ascontiguousarray(
        np.tile(np.vstack([-sinb.T, sinb.T]), (1, B)), dtype=BF16_NP
    )

    # triangular mask for the leading 128 columns of each diagonal tile
    p = np.arange(D)[:, None]
    f = np.arange(D)[None, :]
    masks = (f >= p).astype(BF16_NP)


    def pmajor(w):  # (4096, C) -> (128, KT_w, C) with row r = p + 128k
        kt = w.shape[0] // 128
        return np.ascontiguousarray(w.reshape(kt, 128, w.shape[1]).transpose(1, 0, 2))

    in_maps = []
    for c in range(N_CORES):
        in_maps.append(
            {
                "xTc": xTc,
                "wq": pmajor(w_q[:, c * 512:(c + 1) * 512]),
                "wk": pmajor(w_k[:, c * D:(c + 1) * D]),
                "wv": pmajor(w_v[:, c * D:(c + 1) * D]),
                "wo": pmajor(w_o[c * 512:(c + 1) * 512, :]),
                "cosF": cosF,
                "sinF": sinF,
                "masks": masks,
            }
        )
    return in_maps


def kernel(x, cos_half, sin_half, w_q, w_k, w_v, w_o, trace=None):
    if trace is None:
        trace = os.environ.get("KTRACE", "0") == "1"
    global LAST_RESULT
    in_maps = prep_inputs(x, cos_half, sin_half, w_q, w_k, w_v, w_o)
    res = run_bass_kernel_spmd(
        _nc(), in_maps, core_ids=list(range(N_CORES)), trace=trace
    )
    LAST_RESULT = res
    acc = res.results[0]["outT"].astype(np.float32)
    for c in range(1, N_CORES):
        acc += res.results[c]["outT"].astype(np.float32)
    return np.ascontiguousarray(acc.T).astype(BF16_NP).reshape(B, S, HID)


_NC = None
LAST_RESULT = None


def _nc():
    global _NC
    if _NC is None:
        _NC = build()
    return _NC



# revision 45
# speedup vs baseline: 1.1706x; 1.0074x over previous
"""Trainium2 Bass kernel for nn_Attention_27994596836196.

GQA attention block (B=2, S=2048, HID=4096, 32 q heads / 8 kv groups,
rope, causal, out-projection), tensor-parallel over the 8 NeuronCores of
one TRN2 chip: core c owns q heads 4c..4c+3 and kv group c.  Each core
computes its heads' Q^T/K^T/V projections from a host-pretransposed
activation matrix, runs causal flash-style attention in a transposed
(keys-on-partitions) layout, and contracts its 512-row slice of w_o into
a full-size partial output; the host sums the eight partials
(collectives deliberately avoided: a collective in the NEFF measurably
slows every PE instruction by ~20%).

PE-stream optimizations over the first working version (863us -> ~733us):
- startup: host-prepped partition-major weight layouts, consumption-
  ordered k-chunked loads split across the sync/scalar HWDGE + gpsimd
  SWDGE rings; first matmul at ~14us instead of 46us.
- softmax denominator: p-tiles accumulated on the vector engine in bf16
  with ONE ones-matmul per (head, query-chunk) instead of one per key
  tile (-288 matmuls).
- w_o interleave: attention is exp(ACT)-paced, so w_o m-tile chains for
  the previous query chunk are hand-drained one per attention key-tile
  to keep the PE saturated; final drain rotates all freed PSUM pools.
- causal trim: diagonal key tiles compute only columns >= 128*dd.
- rope: q/k d-columns host-permuted to [evens; odds] so the pair-swap is
  two 64-partition SBUF->SBUF copies instead of a PE matmul per tile.

Self-contained: builds and runs via concourse (bass/tile) from
/opt/trn_rl_repo through bass_utils.run_bass_kernel_spmd on cores 0-7.
"""

import os
import sys

sys.path.insert(0, "/opt/trn_rl_repo")

import numpy as np
import ml_dtypes

# NTFF profiling hook shim: this agent image's antenv package lacks
# axon_hooks, which run_bass_kernel_spmd(trace=True) imports.  Harmless
# when tracing is off; registers the real hook when available.
try:
    import antenv.axon_hooks  # noqa: F401
except ImportError:
    import types

    _m = types.ModuleType("antenv.axon_hooks")
    _m._HOOK = None
    _m.set_axon_ntff_profile_hook = lambda h: setattr(_m, "_HOOK", h)
    _m.get_axon_ntff_profile_hook = lambda: _m._HOOK
    sys.modules["antenv.axon_hooks"] = _m
    try:
        import antenv

        antenv.axon_hooks = _m
        from trn_agent_boot.trn_boot import _ntff_profile_via_ctypes

        _m.set_axon_ntff_profile_hook(
            _ntff_profile_via_ctypes("/opt/axon/libaxon_pjrt.so")
        )
    except Exception:
        pass

import bass_rust
import concourse.bass as bass
import concourse.tile as tile
from concourse import bass_isa
from concourse import mybir
from concourse.bass_utils import run_bass_kernel_spmd
from contextlib import ExitStack

# ---------------------------------------------------------------------------
# Workaround for this walrus build's cap of ONE sync-wait command per
# instruction: Tile's sem-assignment freely attaches several waits to one
# instruction and codegen rejects it ("Too many sync wait commands").
# Split the waits across same-engine NoOps preceding the instruction.
# ---------------------------------------------------------------------------
MAX_WAITS = 1


def split_multi_waits(nc):
    n_split = 0
    for f in nc.m.functions:
        for bb in f.blocks:
            out = []
            for inst in bb.instructions:
                si = inst.sync_info
                if si is not None and si.on_wait and len(si.on_wait) > MAX_WAITS:
                    waits = list(si.on_wait)
                    extra, keep = waits[:-MAX_WAITS], waits[-MAX_WAITS:]
                    for i in range(0, len(extra), MAX_WAITS):
                        nop = bass_rust.InstNoOp(
                            name=f"I-{nc.next_id()}", ins=[], outs=[]
                        )
                        nop.engine = inst.engine
                        nop.sync_info = mybir.SyncInfo(
                            on_wait=extra[i : i + MAX_WAITS], on_update=[]
                        )
                        out.append(nop)
                    si.on_wait = keep
                    n_split += 1
                out.append(inst)
            bb.instructions[:] = out
    return n_split



BF16 = mybir.dt.bfloat16
F32 = mybir.dt.float32

N_CORES = 8
B, S, HID = 2, 2048, 4096
GKV = 8  # kv groups in the full model
BS = B * S  # 4096
D = 128
NH = 4          # q heads per core
KT = HID // 128  # 32 k-tiles
SC = 512        # free-dim chunk
NSC = BS // SC  # 8
SCALE = 1.0 / (D ** 0.5)
EXP = mybir.ActivationFunctionType.Exp
LOG = mybir.ActivationFunctionType.Ln


def build():
    nc = bass.Bass(num_devices=N_CORES)

    # Host-prepped layouts: partition dim first, per-partition contiguous
    # blocks so every HWDGE DMA uses >=1KB descriptors.
    xTc = nc.declare_dram_parameter("xTc", [128, NSC, KT, SC], BF16, isOutput=False)
    wq = nc.declare_dram_parameter("wq", [128, KT, NH * D], BF16, isOutput=False)
    wk = nc.declare_dram_parameter("wk", [128, KT, D], BF16, isOutput=False)
    wv = nc.declare_dram_parameter("wv", [128, KT, D], BF16, isOutput=False)
    wo = nc.declare_dram_parameter("wo", [128, NH, HID], BF16, isOutput=False)
    cosF = nc.declare_dram_parameter("cosF", [D, BS], BF16, isOutput=False)
    sinF = nc.declare_dram_parameter("sinF", [D, BS], BF16, isOutput=False)
    masks = nc.declare_dram_parameter("masks", [D, D], BF16, isOutput=False)
    outT = nc.declare_dram_parameter("outT", [HID, BS], BF16, isOutput=True)

    vT_d = nc.dram_tensor("vT_d", [D, BS], BF16)

    with tile.TileContext(nc, num_cores=N_CORES) as tc, ExitStack() as ctx:
        # ---- long-lived pools -------------------------------------------
        singles = ctx.enter_context(tc.tile_pool(name="singles", bufs=1))
        qkv_sb = ctx.enter_context(tc.tile_pool(name="qkv_sb", bufs=1))
        ps_acc = ctx.enter_context(tc.tile_pool(name="ps_acc", bufs=2, space="PSUM"))
        ps_s = ctx.enter_context(tc.tile_pool(name="ps_s", bufs=3, space="PSUM"))
        ps_l = ctx.enter_context(tc.tile_pool(name="ps_l", bufs=1, space="PSUM"))
        ps_w = ctx.enter_context(tc.tile_pool(name="ps_w", bufs=2, space="PSUM"))

        q_sb = [
            qkv_sb.tile([D, BS], BF16, tag=f"q{h}", name=f"q_sb{h}")
            for h in range(NH)
        ]
        k_sb = qkv_sb.tile([D, BS], BF16, tag="k")
        v_sb = qkv_sb.tile([D, KT, D], BF16, tag="v")  # V natural: [sk_local, j, d]

        # ---- phase 1: projections + rope --------------------------------
        with tc.tile_pool(name="w1", bufs=1) as w1, \
             tc.tile_pool(name="xt", bufs=4) as xtp, \
             tc.tile_pool(name="rope", bufs=4) as rope, \
             tc.tile_pool(name="vt", bufs=3) as vtp:

            # scalar HWDGE queue (idle engine in phase 1): weights in k-chunks,
            # in the order compute consumes them: wk (first projection), wv,
            # then the 4MB wq.
            wk_sb = w1.tile([128, KT, D], BF16, tag="wk")
            wv_sb = w1.tile([128, KT, D], BF16, tag="wv")
            wq_sb = w1.tile([128, KT, NH * D], BF16, tag="wq")
            for g in range(4):
                nc.scalar.dma_start(
                    wk_sb[:, g * 8:(g + 1) * 8, :], wk[:, g * 8:(g + 1) * 8, :]
                )
            for g in range(4):
                nc.scalar.dma_start(
                    wv_sb[:, g * 8:(g + 1) * 8, :], wv[:, g * 8:(g + 1) * 8, :]
                )
            for g in range(4):
                nc.scalar.dma_start(
                    wq_sb[:, g * 8:(g + 1) * 8, :], wq[:, g * 8:(g + 1) * 8, :]
                )
            # gpsimd SWDGE queue: rope tables (needed at the first rope_b,
            # which trails by a full t_i block) and the attention masks.
            cos_sb = singles.tile([D, BS], BF16)
            nc.gpsimd.dma_start(cos_sb[:], cosF[:])
            sin_sb = singles.tile([D, BS], BF16)
            nc.gpsimd.dma_start(sin_sb[:], sinF[:])
            mask_sb = singles.tile([D, D], BF16)
            nc.gpsimd.dma_start(mask_sb[:], masks[:])
            ones_sb = singles.tile([D, D], BF16)
            nc.vector.memset(ones_sb[:], 1.0)

            def rope_a(ps_q, cw):
                qeo = rope.tile([D, cw], BF16, tag="qeo")
                nc.vector.tensor_copy(qeo[:], ps_q[:])
                return qeo

            def rope_b(qeo, dst, col0, cw):
                # d-dims are host-permuted to [evens; odds], so the rope
                # pair-swap is an exchange of the two 64-partition halves:
                # two SBUF->SBUF DMA copies instead of a PE matmul.
                qsw = rope.tile([D, cw], BF16, tag="qsw")
                nc.sync.dma_start(qsw[0:64, :], qeo[64:128, :])
                nc.sync.dma_start(qsw[64:128, :], qeo[0:64, :])
                t1 = rope.tile([D, cw], BF16, tag="t1")
                nc.vector.tensor_mul(t1[:], qeo[:], cos_sb[:, col0:col0 + cw])
                t2 = rope.tile([D, cw], BF16, tag="t2")
                nc.vector.tensor_mul(t2[:], qsw[:], sin_sb[:, col0:col0 + cw])
                nc.vector.tensor_add(dst[:, col0:col0 + cw], t1[:], t2[:])

            def proj_pass(sc, c_lo, cw, nsub):
                # one projection pass over chunk sc's columns [c_lo, c_lo+cw)
                col0 = sc * SC + c_lo
                # sc=0 in fine subs on the sync ring; later chunks prefetch on
                # the scalar ring whose FIFO (wk, wv, wq, xt...) keeps startup
                # HBM bandwidth on the weights until they have landed.
                xt_dma = nc.sync.dma_start if sc == 0 else nc.scalar.dma_start
                xth = []
                for g in range(2):
                    t = xtp.tile([128, KT // 2, cw], BF16, tag="xt")
                    for q in range(nsub):
                        kn = (KT // 2) // nsub
                        xt_dma(
                            t[:, q * kn:(q + 1) * kn, :],
                            xTc[:, sc,
                                g * (KT // 2) + q * kn:
                                g * (KT // 2) + (q + 1) * kn,
                                c_lo:c_lo + cw],
                        )
                    xth.append(t)
                xts = [xth[k // (KT // 2)][:, k % (KT // 2), :] for k in range(KT)]

                # t_i order [k, v, q0..q3]: K first so compute starts on the
                # 1MB wk before the 4MB wq has streamed in.
                pending = None  # deferred rope_b so PE never waits on DVE copy
                for t_i in range(6):
                    ps_t = ps_acc.tile(
                        [D, cw], F32, tag="acc", name=f"ps_t{sc}_{c_lo}_{t_i}"
                    )
                    for k in range(KT):
                        if t_i == 0:
                            lhs = wk_sb[:, k, :]
                        elif t_i == 1:
                            lhs = wv_sb[:, k, :]
                        else:
                            lhs = wq_sb[:, k, bass.ts(t_i - 2, D)]
                        nc.tensor.matmul(
                            ps_t[:], lhs, xts[k],
                            start=(k == 0), stop=(k == KT - 1),
                        )
                    if t_i == 1:
                        vt = vtp.tile([D, cw], BF16, tag="vt")
                        nc.vector.tensor_copy(vt[:], ps_t[:])
                        nc.sync.dma_start(vT_d[:, col0:col0 + cw], vt[:])
                    else:
                        qeo = rope_a(ps_t, cw)
                        if pending is not None:
                            rope_b(*pending)
                        dst = k_sb if t_i == 0 else q_sb[t_i - 2]
                        pending = (qeo, dst, col0, cw)
                rope_b(*pending)

            for sc in range(NSC):
                proj_pass(sc, 0, SC, 4 if sc == 0 else 1)

                # V: read this chunk back transposed -> natural (sk, d) tiles
                # (sync queue: scalar engine must stay clear for phase-2 exps)
                for j in range(4 * sc, 4 * (sc + 1)):
                    nc.sync.dma_start_transpose(
                        v_sb[:, j, :], vT_d[:, bass.ts(j, D)]
                    )

        # ---- phase 2+3: attention with interleaved output projection ----
        # c-outer / h-inner: after all 4 heads finish query-chunk c, the
        # w_o matmuls for that chunk become runnable, and the scheduler
        # uses them to fill the PE while chunk c+1's attention is paced by
        # the scalar-engine exps.  The softmax denominator is accumulated
        # on the vector engine (bf16) with a single ones-matmul per chunk
        # instead of one per key tile.  Diagonal key tiles only compute
        # the columns the causal mask keeps (query cols >= 128*dd).
        with tc.tile_pool(name="pexp", bufs=8) as pexp, \
             tc.tile_pool(name="asml", bufs=2) as asml, \
             tc.tile_pool(name="paccp", bufs=3) as paccp, \
             tc.tile_pool(name="w3", bufs=1) as w3, \
             tc.tile_pool(name="aall", bufs=2) as aallp, \
             tc.tile_pool(name="o3p", bufs=4) as o3p:

            # sync ring, phase-2 priority: keeps the 4MB transfer out of the
            # startup HBM window (it would otherwise race xt/wq for bandwidth)
            wo_sb = w3.tile([128, NH, HID], BF16, tag="wo")
            for g in range(4):
                nc.sync.dma_start(
                    wo_sb[:, :, g * (HID // 4):(g + 1) * (HID // 4)],
                    wo[:, :, g * (HID // 4):(g + 1) * (HID // 4)],
                )

            # No collective: each core contracts only its own 4 heads' A^T
            # (512 of 4096 rows) against its w_o row-slice, producing a full
            # (HID, BS) partial that the host sums across cores.  The m-tile
            # emitters are queued and drained one per attention key-tile so
            # the PE always has w_o work while the exps pace attention.
            wo_fill = []
            fill_state = [0.0, 0.0]  # [accumulator, rate]

            def make_wo_m(b, nl, m, a_all):
                def emit(pool=ps_w, tg="wo"):
                    col = b * S + nl * SC
                    w_ps = pool.tile([D, SC], F32, tag=tg, name=f"wo{b}_{nl}_{m}")
                    for h in range(NH):
                        nc.tensor.matmul(
                            w_ps[:],
                            wo_sb[:, h, bass.ts(m, D)],
                            a_all[h][:, nl * SC:(nl + 1) * SC],
                            start=(h == 0), stop=(h == NH - 1),
                        )
                    ot = o3p.tile([D, SC], BF16, tag="ot", name=f"ot{b}_{nl}_{m}")
                    nc.any.tensor_copy(ot[:], w_ps[:])
                    nc.sync.dma_start(
                        outT[bass.ts(m, D), col:col + SC], ot[:]
                    )
                return emit

            def fill_tick():
                fill_state[0] += fill_state[1]
                while fill_state[0] >= 1.0 and wo_fill:
                    wo_fill.pop(0)()
                    fill_state[0] -= 1.0

            def attention_chunk(b, c, h, a_all):
                qh = q_sb[h]
                sq = b * S + c * SC
                nsk = 4 * (c + 1)
                o_ps = ps_acc.tile([D, SC], F32, tag="acc")
                pacc = paccp.tile([D, SC], BF16, tag="pacc")
                pend = []  # PE lookahead so o-matmuls trail the exps

                def flush(stop):
                    jp, pp, c0 = pend.pop(0)
                    nc.tensor.matmul(
                        o_ps[:, c0:], v_sb[:, b * (S // D) + jp, :], pp[:, c0:],
                        start=(jp == 0), stop=stop,
                    )

                for j in range(nsk):
                    dd = j - 4 * c
                    c0 = 128 * dd if dd > 0 else 0
                    s_ps = ps_s.tile([D, SC], F32, tag="s")
                    nc.tensor.matmul(
                        s_ps[:, c0:],
                        k_sb[:, b * S + j * D: b * S + (j + 1) * D],
                        qh[:, sq + c0:sq + SC],
                        start=True, stop=True,
                    )
                    if len(pend) == 4:
                        flush(False)
                    p_sb = pexp.tile([D, SC], BF16, tag="p")
                    nc.scalar.activation(p_sb[:, c0:], s_ps[:, c0:], EXP, scale=SCALE)
                    if dd >= 0:
                        # triangular mask on the leading 128 columns only
                        nc.vector.tensor_mul(
                            p_sb[:, c0:c0 + D], p_sb[:, c0:c0 + D],
                            mask_sb[:, 0:D],
                        )
                    if j == 0:
                        nc.vector.tensor_copy(pacc[:], p_sb[:])
                    else:
                        nc.vector.tensor_add(
                            pacc[:, c0:], pacc[:, c0:], p_sb[:, c0:]
                        )
                    pend.append((j, p_sb, c0))
                    fill_tick()
                while pend:
                    flush(len(pend) == 1)
                # denominator: single ones-matmul over the DVE-accumulated sum
                l_ps = ps_l.tile([D, SC], F32, tag="l")
                nc.tensor.matmul(l_ps[:], ones_sb[:], pacc[:], start=True, stop=True)
                # 1/l = exp(-ln(l)); ACT reciprocal is banned.
                lg = asml.tile([D, SC], F32, tag="lg")
                nc.scalar.activation(lg[:], l_ps[:], LOG)
                rec = asml.tile([D, SC], F32, tag="rec")
                nc.scalar.activation(rec[:], lg[:], EXP, scale=-1.0)
                nc.vector.tensor_mul(
                    a_all[h][:, c * SC:(c + 1) * SC], o_ps[:], rec[:]
                )

            for b in range(B):
                a_all = [
                    aallp.tile([D, S], BF16, tag=f"a{h}", name=f"a_all{b}_{h}")
                    for h in range(NH)
                ]
                for c in range(S // SC):
                    ntiles = 4 * (c + 1) * NH
                    # start draining a couple of tiles in: the previous
                    # chunk's last rescale is still in the ACT/DVE pipe
                    fill_state[0] = -2.0
                    fill_state[1] = len(wo_fill) / max(1, ntiles - 2)
                    for h in range(NH):
                        attention_chunk(b, c, h, a_all)
                    wo_fill.extend(
                        make_wo_m(b, c, m, a_all) for m in range(KT)
                    )
            # final drain: attention is done, so rotate through the freed
            # attention PSUM pools to avoid bank-reuse stalls
            drain_pools = [(ps_w, "wo"), (ps_s, "s"), (ps_acc, "acc"), (ps_l, "l")]
            di = 0
            while wo_fill:
                pool, tg = drain_pools[di % len(drain_pools)]
                di += 1
                wo_fill.pop(0)(pool, tg)

    split_multi_waits(nc)
    return nc


BF16_NP = ml_dtypes.bfloat16


def prep_inputs(x, cos_half, sin_half, w_q, w_k, w_v, w_o):
    x = np.asarray(x)
    cos_half = np.asarray(cos_half, dtype=np.float32)
    sin_half = np.asarray(sin_half, dtype=np.float32)
    w_q, w_k, w_v, w_o = (np.asarray(a) for a in (w_q, w_k, w_v, w_o))

    X = x.reshape(B * S, HID)
    xT = np.ascontiguousarray(X.T)  # (HID, BS) bf16
    # chunk-major layout: xTc[p, sc, k, s] = xT[p + 128k, sc*512 + s] so a
    # phase-1 tile DMA reads one contiguous 16-32KB block per partition
    xTc = np.ascontiguousarray(
        xT.reshape(KT, 128, NSC, SC).transpose(1, 2, 0, 3)
    )

    # d-dims of q/k are permuted to [evens; odds] (scores are invariant since
    # q and k share the permutation; V and w_o are untouched).  In that
    # layout rope's pair-swap is a swap of the 64-partition halves, and the
    # per-row tables are [cos; cos] and [-sin; sin].
    perm = np.concatenate([np.arange(0, D, 2), np.arange(1, D, 2)])
    w_q = np.ascontiguousarray(w_q.reshape(HID, HID // D, D)[:, :, perm].reshape(HID, HID))
    w_k = np.ascontiguousarray(w_k.reshape(HID, GKV, D)[:, :, perm].reshape(HID, GKV * D))

    cosb = cos_half.astype(BF16_NP)  # reference casts cos/sin to bf16 in _rope
    sinb = sin_half.astype(BF16_NP)
    cosF = np.ascontiguousarray(
        np.tile(np.vstack([cosb.T, cosb.T]), (1, B)), dtype=BF16_NP
    )
    sinF = np.ascontiguousarray(
        np.tile(np.vstack([-sinb.T, sinb.T]), (1, B)), dtype=BF16_NP
    )

    # triangular mask for the leading 128 columns of each diagonal tile
    p = np.arange(D)[:, None]
    f = np.arange(D)[None, :]
    masks = (f >= p).astype(BF16_NP)


    def pmajor(w):  # (4096, C) -> (128, KT_w, C) with row r = p + 128k
        kt = w.shape[0] // 128
        return np.ascontiguousarray(w.reshape(kt, 128, w.shape[1]).transpose(1, 0, 2))

    in_maps = []
    for c in range(N_CORES):
        in_maps.append(
            {
                "xTc": xTc,
                "wq": pmajor(w_q[:, c * 512:(c + 1) * 512]),
                "wk": pmajor(w_k[:, c * D:(c + 1) * D]),
                "wv": pmajor(w_v[:, c * D:(c + 1) * D]),
                "wo": pmajor(w_o[c * 512:(c + 1) * 512, :]),
                "cosF": cosF,
                "sinF": sinF,
                "masks": masks,
            }
        )
    return in_maps


def kernel(x, cos_half, sin_half, w_q, w_k, w_v, w_o, trace=None):
    if trace is None:
        trace = os.environ.get("KTRACE", "0") == "1"
    global LAST_RESULT
    in_maps = prep_inputs(x, cos_half, sin_half, w_q, w_k, w_v, w_o)
    res = run_bass_kernel_spmd(
        _nc(), in_maps, core_ids=list(range(N_CORES)), trace=trace
    )
    LAST_RESULT = res
    acc = res.results[0]["outT"].astype(np.float32)
    for c in range(1, N_CORES):
        acc += res.results[c]["outT"].astype(np.float32)
    return np.ascontiguousarray(acc.T).astype(BF16_NP).reshape(B, S, HID)


_NC = None
LAST_RESULT = None


def _nc():
    global _NC
    if _NC is None:
        _NC = build()
    return _NC



# revision 47
# speedup vs baseline: 1.1883x; 1.0151x over previous
"""Trainium2 Bass kernel for nn_Attention_27994596836196.

GQA attention block (B=2, S=2048, HID=4096, 32 q heads / 8 kv groups,
rope, causal, out-projection), tensor-parallel over the 8 NeuronCores of
one TRN2 chip: core c owns q heads 4c..4c+3 and kv group c.  Each core
computes its heads' Q^T/K^T/V projections from a host-pretransposed
activation matrix, runs causal flash-style attention in a transposed
(keys-on-partitions) layout, and contracts its 512-row slice of w_o into
a full-size partial output; the host sums the eight partials
(collectives deliberately avoided: a collective in the NEFF measurably
slows every PE instruction by ~20%).

PE-stream optimizations over the first working version (863us -> ~733us):
- startup: host-prepped partition-major weight layouts, consumption-
  ordered k-chunked loads split across the sync/scalar HWDGE + gpsimd
  SWDGE rings; first matmul at ~14us instead of 46us.
- softmax denominator: p-tiles accumulated on the vector engine in bf16
  with ONE ones-matmul per (head, query-chunk) instead of one per key
  tile (-288 matmuls).
- w_o interleave: attention is exp(ACT)-paced, so w_o m-tile chains for
  the previous query chunk are hand-drained one per attention key-tile
  to keep the PE saturated; final drain rotates all freed PSUM pools.
- causal trim: diagonal key tiles compute only columns >= 128*dd.
- rope: q/k d-columns host-permuted to [evens; odds] so the pair-swap is
  two 64-partition SBUF->SBUF copies instead of a PE matmul per tile.

Self-contained: builds and runs via concourse (bass/tile) from
/opt/trn_rl_repo through bass_utils.run_bass_kernel_spmd on cores 0-7.
"""

import os
import sys

sys.path.insert(0, "/opt/trn_rl_repo")

import numpy as np
import ml_dtypes

# NTFF profiling hook shim: this agent image's antenv package lacks
# axon_hooks, which run_bass_kernel_spmd(trace=True) imports.  Harmless
# when tracing is off; registers the real hook when available.
try:
    import antenv.axon_hooks  # noqa: F401
except ImportError:
    import types

    _m = types.ModuleType("antenv.axon_hooks")
    _m._HOOK = None
    _m.set_axon_ntff_profile_hook = lambda h: setattr(_m, "_HOOK", h)
    _m.get_axon_ntff_profile_hook = lambda: _m._HOOK
    sys.modules["antenv.axon_hooks"] = _m
    try:
        import antenv

        antenv.axon_hooks = _m
        from trn_agent_boot.trn_boot import _ntff_profile_via_ctypes

        _m.set_axon_ntff_profile_hook(
            _ntff_profile_via_ctypes("/opt/axon/libaxon_pjrt.so")
        )
    except Exception:
        pass

import bass_rust
import concourse.bass as bass
import concourse.tile as tile
from concourse import bass_isa
from concourse import mybir
from concourse.bass_utils import run_bass_kernel_spmd
from contextlib import ExitStack

# ---------------------------------------------------------------------------
# Workaround for this walrus build's cap of ONE sync-wait command per
# instruction: Tile's sem-assignment freely attaches several waits to one
# instruction and codegen rejects it ("Too many sync wait commands").
# Split the waits across same-engine NoOps preceding the instruction.
# ---------------------------------------------------------------------------
MAX_WAITS = 1


def split_multi_waits(nc):
    n_split = 0
    for f in nc.m.functions:
        for bb in f.blocks:
            out = []
            for inst in bb.instructions:
                si = inst.sync_info
                if si is not None and si.on_wait and len(si.on_wait) > MAX_WAITS:
                    waits = list(si.on_wait)
                    extra, keep = waits[:-MAX_WAITS], waits[-MAX_WAITS:]
                    for i in range(0, len(extra), MAX_WAITS):
                        nop = bass_rust.InstNoOp(
                            name=f"I-{nc.next_id()}", ins=[], outs=[]
                        )
                        nop.engine = inst.engine
                        nop.sync_info = mybir.SyncInfo(
                            on_wait=extra[i : i + MAX_WAITS], on_update=[]
                        )
                        out.append(nop)
                    si.on_wait = keep
                    n_split += 1
                out.append(inst)
            bb.instructions[:] = out
    return n_split



BF16 = mybir.dt.bfloat16
F32 = mybir.dt.float32

N_CORES = 8
B, S, HID = 2, 2048, 4096
GKV = 8  # kv groups in the full model
BS = B * S  # 4096
D = 128
NH = 4          # q heads per core
KT = HID // 128  # 32 k-tiles
SC = 512        # free-dim chunk
NSC = BS // SC  # 8
SCALE = 1.0 / (D ** 0.5)
EXP = mybir.ActivationFunctionType.Exp
LOG = mybir.ActivationFunctionType.Ln


def build():
    nc = bass.Bass(num_devices=N_CORES)

    # Host-prepped layouts: partition dim first, per-partition contiguous
    # blocks so every HWDGE DMA uses >=1KB descriptors.
    xTc = nc.declare_dram_parameter("xTc", [128, NSC, KT, SC], BF16, isOutput=False)
    wq = nc.declare_dram_parameter("wq", [128, KT, NH * D], BF16, isOutput=False)
    wk = nc.declare_dram_parameter("wk", [128, KT, D], BF16, isOutput=False)
    wv = nc.declare_dram_parameter("wv", [128, KT, D], BF16, isOutput=False)
    wo = nc.declare_dram_parameter("wo", [128, NH, HID], BF16, isOutput=False)
    cosF = nc.declare_dram_parameter("cosF", [D, BS], BF16, isOutput=False)
    sinF = nc.declare_dram_parameter("sinF", [D, BS], BF16, isOutput=False)
    masks = nc.declare_dram_parameter("masks", [D, D], BF16, isOutput=False)
    outT = nc.declare_dram_parameter("outT", [HID, BS], BF16, isOutput=True)

    vT_d = nc.dram_tensor("vT_d", [D, BS], BF16)

    with tile.TileContext(nc, num_cores=N_CORES) as tc, ExitStack() as ctx:
        # ---- long-lived pools -------------------------------------------
        singles = ctx.enter_context(tc.tile_pool(name="singles", bufs=1))
        qkv_sb = ctx.enter_context(tc.tile_pool(name="qkv_sb", bufs=1))
        ps_acc = ctx.enter_context(tc.tile_pool(name="ps_acc", bufs=2, space="PSUM"))
        ps_s = ctx.enter_context(tc.tile_pool(name="ps_s", bufs=3, space="PSUM"))
        ps_l = ctx.enter_context(tc.tile_pool(name="ps_l", bufs=1, space="PSUM"))
        ps_w = ctx.enter_context(tc.tile_pool(name="ps_w", bufs=2, space="PSUM"))

        q_sb = [
            qkv_sb.tile([D, BS], BF16, tag=f"q{h}", name=f"q_sb{h}")
            for h in range(NH)
        ]
        k_sb = qkv_sb.tile([D, BS], BF16, tag="k")
        v_sb = qkv_sb.tile([D, KT, D], BF16, tag="v")  # V natural: [sk_local, j, d]

        # ---- phase 1: projections + rope --------------------------------
        with tc.tile_pool(name="w1", bufs=1) as w1, \
             tc.tile_pool(name="xt", bufs=4) as xtp, \
             tc.tile_pool(name="rope", bufs=4) as rope, \
             tc.tile_pool(name="vt", bufs=3) as vtp:

            # scalar HWDGE queue (idle engine in phase 1): weights in k-chunks,
            # in the order compute consumes them: wk (first projection), wv,
            # then the 4MB wq.
            wk_sb = w1.tile([128, KT, D], BF16, tag="wk")
            wv_sb = w1.tile([128, KT, D], BF16, tag="wv")
            wq_sb = w1.tile([128, KT, NH * D], BF16, tag="wq")
            for g in range(4):
                nc.scalar.dma_start(
                    wk_sb[:, g * 8:(g + 1) * 8, :], wk[:, g * 8:(g + 1) * 8, :]
                )
            for g in range(4):
                nc.scalar.dma_start(
                    wv_sb[:, g * 8:(g + 1) * 8, :], wv[:, g * 8:(g + 1) * 8, :]
                )
            for g in range(4):
                nc.scalar.dma_start(
                    wq_sb[:, g * 8:(g + 1) * 8, :], wq[:, g * 8:(g + 1) * 8, :]
                )
            # gpsimd SWDGE queue: rope tables (needed at the first rope_b,
            # which trails by a full t_i block) and the attention masks.
            cos_sb = singles.tile([D, BS], BF16)
            nc.gpsimd.dma_start(cos_sb[:], cosF[:])
            sin_sb = singles.tile([D, BS], BF16)
            nc.gpsimd.dma_start(sin_sb[:], sinF[:])
            mask_sb = singles.tile([D, D], BF16)
            nc.gpsimd.dma_start(mask_sb[:], masks[:])
            ones_sb = singles.tile([D, D], BF16)
            nc.vector.memset(ones_sb[:], 1.0)

            def rope_a(ps_q, cw):
                qeo = rope.tile([D, cw], BF16, tag="qeo")
                nc.vector.tensor_copy(qeo[:], ps_q[:])
                return qeo

            def rope_b(qeo, dst, col0, cw):
                # d-dims are host-permuted to [evens; odds], so the rope
                # pair-swap is an exchange of the two 64-partition halves:
                # two SBUF->SBUF DMA copies instead of a PE matmul.
                qsw = rope.tile([D, cw], BF16, tag="qsw")
                nc.sync.dma_start(qsw[0:64, :], qeo[64:128, :])
                nc.sync.dma_start(qsw[64:128, :], qeo[0:64, :])
                t1 = rope.tile([D, cw], BF16, tag="t1")
                nc.vector.tensor_mul(t1[:], qeo[:], cos_sb[:, col0:col0 + cw])
                t2 = rope.tile([D, cw], BF16, tag="t2")
                nc.vector.tensor_mul(t2[:], qsw[:], sin_sb[:, col0:col0 + cw])
                nc.vector.tensor_add(dst[:, col0:col0 + cw], t1[:], t2[:])

            def proj_pass(sc, c_lo, cw, nsub):
                # one projection pass over chunk sc's columns [c_lo, c_lo+cw)
                col0 = sc * SC + c_lo
                # sc=0 in fine subs on the sync ring; later chunks prefetch on
                # the scalar ring whose FIFO (wk, wv, wq, xt...) keeps startup
                # HBM bandwidth on the weights until they have landed.
                xt_dma = nc.sync.dma_start if sc == 0 else nc.scalar.dma_start
                xth = []
                for g in range(2):
                    t = xtp.tile([128, KT // 2, cw], BF16, tag="xt")
                    for q in range(nsub):
                        kn = (KT // 2) // nsub
                        xt_dma(
                            t[:, q * kn:(q + 1) * kn, :],
                            xTc[:, sc,
                                g * (KT // 2) + q * kn:
                                g * (KT // 2) + (q + 1) * kn,
                                c_lo:c_lo + cw],
                        )
                    xth.append(t)
                xts = [xth[k // (KT // 2)][:, k % (KT // 2), :] for k in range(KT)]

                # t_i order [k, v, q0..q3]: K first so compute starts on the
                # 1MB wk before the 4MB wq has streamed in.
                pending = None  # deferred rope_b so PE never waits on DVE copy
                for t_i in range(6):
                    ps_t = ps_acc.tile(
                        [D, cw], F32, tag="acc", name=f"ps_t{sc}_{c_lo}_{t_i}"
                    )
                    for k in range(KT):
                        if t_i == 0:
                            lhs = wk_sb[:, k, :]
                        elif t_i == 1:
                            lhs = wv_sb[:, k, :]
                        else:
                            lhs = wq_sb[:, k, bass.ts(t_i - 2, D)]
                        nc.tensor.matmul(
                            ps_t[:], lhs, xts[k],
                            start=(k == 0), stop=(k == KT - 1),
                        )
                    if t_i == 1:
                        vt = vtp.tile([D, cw], BF16, tag="vt")
                        nc.vector.tensor_copy(vt[:], ps_t[:])
                        nc.sync.dma_start(vT_d[:, col0:col0 + cw], vt[:])
                    else:
                        qeo = rope_a(ps_t, cw)
                        if pending is not None:
                            rope_b(*pending)
                        dst = k_sb if t_i == 0 else q_sb[t_i - 2]
                        pending = (qeo, dst, col0, cw)
                rope_b(*pending)

            for sc in range(NSC):
                proj_pass(sc, 0, SC, 4 if sc == 0 else 1)

                # V: read this chunk back transposed -> natural (sk, d) tiles
                # (sync queue: scalar engine must stay clear for phase-2 exps)
                for j in range(4 * sc, 4 * (sc + 1)):
                    nc.sync.dma_start_transpose(
                        v_sb[:, j, :], vT_d[:, bass.ts(j, D)]
                    )

        # ---- phase 2+3: attention with interleaved output projection ----
        # c-outer / h-inner: after all 4 heads finish query-chunk c, the
        # w_o matmuls for that chunk become runnable, and the scheduler
        # uses them to fill the PE while chunk c+1's attention is paced by
        # the scalar-engine exps.  The softmax denominator is accumulated
        # on the vector engine (bf16) with a single ones-matmul per chunk
        # instead of one per key tile.  Diagonal key tiles only compute
        # the columns the causal mask keeps (query cols >= 128*dd).
        with tc.tile_pool(name="pexp", bufs=8) as pexp, \
             tc.tile_pool(name="asml", bufs=2) as asml, \
             tc.tile_pool(name="paccp", bufs=3) as paccp, \
             tc.tile_pool(name="w3", bufs=1) as w3, \
             tc.tile_pool(name="aall", bufs=2) as aallp, \
             tc.tile_pool(name="o3p", bufs=4) as o3p:

            # sync ring, phase-2 priority: keeps the 4MB transfer out of the
            # startup HBM window (it would otherwise race xt/wq for bandwidth)
            wo_sb = w3.tile([128, NH, HID], BF16, tag="wo")
            for g in range(4):
                nc.sync.dma_start(
                    wo_sb[:, :, g * (HID // 4):(g + 1) * (HID // 4)],
                    wo[:, :, g * (HID // 4):(g + 1) * (HID // 4)],
                )

            # No collective: each core contracts only its own 4 heads' A^T
            # (512 of 4096 rows) against its w_o row-slice, producing a full
            # (HID, BS) partial that the host sums across cores.  The m-tile
            # emitters are queued and drained one per attention key-tile so
            # the PE always has w_o work while the exps pace attention.
            wo_fill = []
            fill_state = [0.0, 0.0]  # [accumulator, rate]

            def make_wo_m(b, nl, m, a_all):
                def emit(pool=ps_w, tg="wo"):
                    col = b * S + nl * SC
                    w_ps = pool.tile([D, SC], F32, tag=tg, name=f"wo{b}_{nl}_{m}")
                    for h in range(NH):
                        nc.tensor.matmul(
                            w_ps[:],
                            wo_sb[:, h, bass.ts(m, D)],
                            a_all[h][:, nl * SC:(nl + 1) * SC],
                            start=(h == 0), stop=(h == NH - 1),
                        )
                    ot = o3p.tile([D, SC], BF16, tag="ot", name=f"ot{b}_{nl}_{m}")
                    nc.any.tensor_copy(ot[:], w_ps[:])
                    nc.sync.dma_start(
                        outT[bass.ts(m, D), col:col + SC], ot[:]
                    )
                return emit

            def fill_tick():
                fill_state[0] += fill_state[1]
                while fill_state[0] >= 1.0 and wo_fill:
                    wo_fill.pop(0)()
                    fill_state[0] -= 1.0

            def attention_chunk(b, c, h, a_all):
                qh = q_sb[h]
                sq = b * S + c * SC
                nsk = 4 * (c + 1)
                o_ps = ps_acc.tile([D, SC], F32, tag="acc")
                pacc = paccp.tile([D, SC], BF16, tag="pacc")
                pend = []  # PE lookahead so o-matmuls trail the exps

                def flush(stop):
                    jp, pp, c0 = pend.pop(0)
                    nc.tensor.matmul(
                        o_ps[:, c0:], v_sb[:, b * (S // D) + jp, :], pp[:, c0:],
                        start=(jp == 0), stop=stop,
                    )

                for j in range(nsk):
                    dd = j - 4 * c
                    c0 = 128 * dd if dd > 0 else 0
                    s_ps = ps_s.tile([D, SC], F32, tag="s")
                    nc.tensor.matmul(
                        s_ps[:, c0:],
                        k_sb[:, b * S + j * D: b * S + (j + 1) * D],
                        qh[:, sq + c0:sq + SC],
                        start=True, stop=True,
                    )
                    if len(pend) == 5:
                        flush(False)
                    p_sb = pexp.tile([D, SC], BF16, tag="p")
                    nc.scalar.activation(p_sb[:, c0:], s_ps[:, c0:], EXP, scale=SCALE)
                    if dd >= 0:
                        # triangular mask on the leading 128 columns only
                        nc.vector.tensor_mul(
                            p_sb[:, c0:c0 + D], p_sb[:, c0:c0 + D],
                            mask_sb[:, 0:D],
                        )
                    if j == 0:
                        nc.vector.tensor_copy(pacc[:], p_sb[:])
                    else:
                        nc.vector.tensor_add(
                            pacc[:, c0:], pacc[:, c0:], p_sb[:, c0:]
                        )
                    pend.append((j, p_sb, c0))
                    fill_tick()
                while pend:
                    flush(len(pend) == 1)
                # denominator: single ones-matmul over the DVE-accumulated sum
                l_ps = ps_l.tile([D, SC], F32, tag="l")
                nc.tensor.matmul(l_ps[:], ones_sb[:], pacc[:], start=True, stop=True)
                # 1/l = exp(-ln(l)); ACT reciprocal is banned.
                lg = asml.tile([D, SC], F32, tag="lg")
                nc.scalar.activation(lg[:], l_ps[:], LOG)
                rec = asml.tile([D, SC], F32, tag="rec")
                nc.scalar.activation(rec[:], lg[:], EXP, scale=-1.0)
                nc.vector.tensor_mul(
                    a_all[h][:, c * SC:(c + 1) * SC], o_ps[:], rec[:]
                )

            for b in range(B):
                a_all = [
                    aallp.tile([D, S], BF16, tag=f"a{h}", name=f"a_all{b}_{h}")
                    for h in range(NH)
                ]
                for c in range(S // SC):
                    ntiles = 4 * (c + 1) * NH
                    # start draining a few tiles in: the previous chunk's
                    # last rescale is still in the ACT/DVE pipe.  c==1 is
                    # the tightest race (highest fill rate, freshest
                    # rescale) and stalls long enough to re-throttle HAM,
                    # so it gets extra headroom.
                    lead = 6.0 if c == 1 else 2.0
                    fill_state[0] = -lead
                    fill_state[1] = len(wo_fill) / max(1.0, ntiles - lead)
                    for h in range(NH):
                        attention_chunk(b, c, h, a_all)
                    wo_fill.extend(
                        make_wo_m(b, c, m, a_all) for m in range(KT)
                    )
            # final drain: attention is done, so rotate through the freed
            # attention PSUM pools to avoid bank-reuse stalls
            drain_pools = [(ps_w, "wo"), (ps_s, "s"), (ps_acc, "acc"), (ps_l, "l")]
            di = 0
            while wo_fill:
                pool, tg = drain_pools[di % len(drain_pools)]
                di += 1
                wo_fill.pop(0)(pool, tg)

    split_multi_waits(nc)
    return nc


BF16_NP = ml_dtypes.bfloat16


def prep_inputs(x, cos_half, sin_half, w_q, w_k, w_v, w_o):
    x = np.asarray(x)
    cos_half = np.asarray(cos_half, dtype=np.float32)
    sin_half = np.asarray(sin_half, dtype=np.float32)
    w_q, w_k, w_v, w_o = (np.asarray(a) for a in (w_q, w_k, w_v, w_o))

    X = x.reshape(B * S, HID)
    xT = np.ascontiguousarray(X.T)  # (HID, BS) bf16
    # chunk-major layout: xTc[p, sc, k, s] = xT[p + 128k, sc*512 + s] so a
    # phase-1 tile DMA reads one contiguous 16-32KB block per partition
    xTc = np.ascontiguousarray(
        xT.reshape(KT, 128, NSC, SC).transpose(1, 2, 0, 3)
    )

    # d-dims of q/k are permuted to [evens; odds] (scores are invariant since
    # q and k share the permutation; V and w_o are untouched).  In that
    # layout rope's pair-swap is a swap of the 64-partition halves, and the
    # per-row tables are [cos; cos] and [-sin; sin].
    perm = np.concatenate([np.arange(0, D, 2), np.arange(1, D, 2)])
    w_q = np.ascontiguousarray(w_q.reshape(HID, HID // D, D)[:, :, perm].reshape(HID, HID))
    w_k = np.ascontiguousarray(w_k.reshape(HID, GKV, D)[:, :, perm].reshape(HID, GKV * D))

    cosb = cos_half.astype(BF16_NP)  # reference casts cos/sin to bf16 in _rope
    sinb = sin_half.astype(BF16_NP)
    cosF = np.ascontiguousarray(
        np.tile(np.vstack([cosb.T, cosb.T]), (1, B)), dtype=BF16_NP
    )
    sinF = np.ascontiguousarray(
        np.tile(np.vstack([-sinb.T, sinb.T]), (1, B)), dtype=BF16_NP
    )

    # triangular mask for the leading 128 columns of each diagonal tile
    p = np.arange(D)[:, None]
    f = np.arange(D)[None, :]
    masks = (f >= p).astype(BF16_NP)


    def pmajor(w):  # (4096, C) -> (128, KT_w, C) with row r = p + 128k
        kt = w.shape[0] // 128
        return np.ascontiguousarray(w.reshape(kt, 128, w.shape[1]).transpose(1, 0, 2))

    in_maps = []
    for c in range(N_CORES):
        in_maps.append(
            {
                "xTc": xTc,
                "wq": pmajor(w_q[:, c * 512:(c + 1) * 512]),
                "wk": pmajor(w_k[:, c * D:(c + 1) * D]),
                "wv": pmajor(w_v[:, c * D:(c + 1) * D]),
                "wo": pmajor(w_o[c * 512:(c + 1) * 512, :]),
                "cosF": cosF,
                "sinF": sinF,
                "masks": masks,
            }
        )
    return in_maps


def kernel(x, cos_half, sin_half, w_q, w_k, w_v, w_o, trace=None):
    if trace is None:
        trace = os.environ.get("KTRACE", "0") == "1"
    global LAST_RESULT
    in_maps = prep_inputs(x, cos_half, sin_half, w_q, w_k, w_v, w_o)
    res = run_bass_kernel_spmd(
        _nc(), in_maps, core_ids=list(range(N_CORES)), trace=trace
    )
    LAST_RESULT = res
    acc = res.results[0]["outT"].astype(np.float32)
    for c in range(1, N_CORES):
        acc += res.results[c]["outT"].astype(np.float32)
    return np.ascontiguousarray(acc.T).astype(BF16_NP).reshape(B, S, HID)


_NC = None
LAST_RESULT = None


def _nc():
    global _NC
    if _NC is None:
        _NC = build()
    return _NC

